# revision 34
# baseline (speedup 1.0000x reference)
"""MAE decoder forward on 8 Trainium2 NeuronCores, data-parallel over batch.

Layout strategy (per core, 4 batches of the 32):
  - Tokens are padded 196 -> 208 per sequence (multiple of 16 for the DMA
    crossbar transpose). Padded token columns carry garbage but are never
    read by any consumer that feeds real outputs.
  - Residual stream: token-major f32 `u` tiles for the pre-LN sums;
    LayerNorm outputs bf16 `xbf` tiles (GEMM sources, residual inputs and
    DMA-transpose sources).
  - Feature-major views XT/Q^T/K^T/OT [128, 4, 832] bf16 come from
    SBUF->SBUF DMA crossbar transposes (no PE transposes). Transposes and
    the output ride the Activation-engine DMA queue; weight streams ride
    the SP queue. All weights are prefetched one full layer ahead with
    two-layer-deep pools so no DMA ever blocks the queue head.
  - All GEMMs run in bf16 (1 cycle/row on the PE).
  - Attention: S^T = K Q^T per head (keys on partitions); exp on ScalarE
    does two heads per instruction (paired PSUM tiles). V carries a ones
    column per head (33 cols/head); AV computes o^T = P^T V token-major
    (moving dim 33, queries on PSUM partitions) so the softmax denominator
    lands in column 32 and normalization is a per-partition reciprocal +
    tensor_scalar. A DMA transpose rebuilds the feature-major attention
    output for the output projection.
  - The whole layer is emitted as a software-pipelined schedule, because
    engine queues execute in order:
      P0: [QK gemm chunk1 + V b2,b3]  zip  [attention b0,b1]
      P1: oproj c0 + residual + LN1(c0) + FFN-transpose c0
      P2: [FFN chunk0]                zip  [attention b2,b3]
      P3: oproj c1 + LN1(c1) + T2(c1); FFN resid c0 + LN2(c0) + Tnext(c0)
      P4: [FFN chunk1]                zip  [next-layer QK gemm c0 + V b0,b1]
      P5: FFN resid c1 + LN2(c1) + Tnext(c1)
    so ScalarE-heavy attention overlaps PE-heavy FFN/GEMM work, and the
    next layer's Q/K/V start before the current layer finishes.
  - LayerNorm: bn_stats/bn_aggr per tile, batched rstd via one ScalarE
    Sqrt + one DVE reciprocal per chunk (avoids activation-table thrash).
"""

import numpy as np
import ml_dtypes

import concourse.bass as bass
import concourse.tile as tile
from concourse import bacc, mybir
from concourse.bass_utils import run_bass_kernel_spmd

F32 = mybir.dt.float32
BF16 = mybir.dt.bfloat16

N = 196
NP = 208  # padded tokens (multiple of 16 for DMA transpose)
D = 512
H = 16
HD = 32
FF = 2048
LN_EPS = 1e-5
N_CORES = 8
B_FULL = 32
BC = B_FULL // N_CORES
TCP = BC * NP  # padded tokens per core (832)
NCH = TCP // 2  # feature-major GEMM moving chunk (416)
NF = FF // 128  # FFN hidden tiles (16)

# token tiles within one sequence: (j, offset, real size, padded size)
TJ = [(0, 0, 128, 128), (1, 128, 68, 80)]
TT = [(b, j, off, sz, szp) for b in range(BC) for (j, off, sz, szp) in TJ]
KJT = ((0, 0, 128), (1, 128, 68))  # attention key tiles
QJT = ((0, 0, 128), (1, 128, 80))  # attention query tiles (padded)


def _r(h, off, ap):
    """Raw element-strided AP into DRAM handle h."""
    return bass.AP(h, off, ap)


def _zip_emit(*gens):
    """Round-robin-advance emission generators until all are exhausted.

    Each entry is a generator or a (generator, weight) pair; weight = number
    of units advanced per round."""
    alive = []
    for g in gens:
        if isinstance(g, tuple):
            alive.append([g[0], g[1]])
        else:
            alive.append([g, 1])
    while alive:
        for ent in list(alive):
            g, w = ent
            for _ in range(w):
                try:
                    next(g)
                except StopIteration:
                    alive.remove(ent)
                    break


def build_decoder(tc, outs, ins, meta):
    nc = tc.nc
    L = meta["L"]

    xe = ins["xe"]  # [BC, 50, 512] f32
    idxf = ins["idxf"]  # [BC, 196] f32
    maskt = ins["maskt"]  # [512] f32
    pe = ins["pe"]  # [196, 512] f32
    iota2 = ins["iota2"]  # [128, 2] f32
    Wq, Wk, Wv, Wo = ins["Wq"], ins["Wk"], ins["Wv"], ins["Wo"]  # bf16
    W1, W2 = ins["W1"], ins["W2"]  # bf16
    y = outs["y"]  # [BC, 196, 512] f32

    import contextlib

    with contextlib.ExitStack() as ctx:
        pc = ctx.enter_context(tc.tile_pool(name="consts", bufs=1))
        pxbf = ctx.enter_context(tc.tile_pool(name="xbf", bufs=2))
        px = ctx.enter_context(tc.tile_pool(name="resid", bufs=3))
        pxt = ctx.enter_context(tc.tile_pool(name="xt", bufs=2))
        pqk = ctx.enter_context(tc.tile_pool(name="qk", bufs=1))
        patt = ctx.enter_context(tc.tile_pool(name="att", bufs=4))
        psm = ctx.enter_context(tc.tile_pool(name="sm", bufs=2))
        pst = ctx.enter_context(tc.tile_pool(name="st", bufs=2))
        plns = ctx.enter_context(tc.tile_pool(name="lnsc", bufs=10))
        ph = ctx.enter_context(tc.tile_pool(name="hp", bufs=3))
        pw = ctx.enter_context(tc.tile_pool(name="wc", bufs=28))
        pw1 = ctx.enter_context(tc.tile_pool(name="w1", bufs=6))
        pw2 = ctx.enter_context(tc.tile_pool(name="w2", bufs=14))
        pg = ctx.enter_context(tc.tile_pool(name="gp", bufs=1))
        # PSUM: 2 banks held by FFN2 accumulators, 4 for the paired score
        # tiles (2 banks each, bank-aligned halves so the two tile_position
        # configs never share a bank), 2 rotating for everything else
        pph = ctx.enter_context(tc.tile_pool(name="psh", bufs=2, space="PSUM"))
        ppsp = ctx.enter_context(tc.tile_pool(name="psp", bufs=2, space="PSUM"))
        ppr = ctx.enter_context(tc.tile_pool(name="psr", bufs=2, space="PSUM"))

        def psum(shape):
            return ppr.tile(shape, F32, tag="ps", name="pst")

        # ---- constants ----
        iota_sb = pc.tile([128, 2], F32, tag="iota")
        nc.sync.dma_start(out=iota_sb, in_=iota2)
        rsq_k = pc.tile([128, 4], mybir.dt.int32, tag="rsqk")
        nc.vector.memset(rsq_k, 0x5F3759DF)
        pe_tm = pc.tile([128, 2, D], BF16, tag="pe")
        nc.vector.memset(pe_tm, 0.0)
        for j, off, sz, szp in TJ:
            nc.sync.dma_start(out=pe_tm[:sz, j, :], in_=pe[off : off + sz, :])

        # ---- prologue: unshuffle gather + pos embed ----
        # f32 copy (residual source) + bf16 copy (GEMM/transpose source)
        rstd0 = pc.tile([128, 4], F32, tag="rstd0")
        nc.vector.memset(rstd0, 1.0)
        mrstd0 = pc.tile([128, 4], F32, tag="mrstd0")
        nc.vector.memset(mrstd0, 0.0)
        u0 = px.tile([128, BC, 2, D], F32, tag="x")
        sc0 = {b: (rstd0, mrstd0, 0) for b in range(BC)}
        x_in = pxbf.tile([128, BC, 2, D], BF16, tag="xbf")
        for b in range(BC):
            sh = pg.tile([128, 2, D], BF16, tag="sh")
            nc.sync.dma_start(out=sh[:49, 0, :], in_=xe[b, 1:50, :])
            nc.sync.dma_start(
                out=sh[49:128, 0, :], in_=_r(maskt.tensor, 0, [[0, 79], [1, D]])
            )
            nc.sync.dma_start(
                out=sh[:68, 1, :], in_=_r(maskt.tensor, 0, [[0, 68], [1, D]])
            )
            idxb = pg.tile([128, NP], F32, tag="idxb")
            nc.vector.memset(idxb[:, N:], -1.0)
            nc.sync.dma_start(
                out=idxb[:, :N], in_=_r(idxf.tensor, b * N, [[0, 128], [1, N]])
            )
            # ptg[p, k, n] = 1.0 if idx[n] == k*128 + p else 0.0 (pad cols 0)
            ptg = pg.tile([128, 2, NP], BF16, tag="ptg")
            for k in range(2):
                nc.vector.tensor_scalar(
                    out=ptg[:, k, :],
                    in0=idxb,
                    scalar1=iota_sb[:, k : k + 1],
                    scalar2=None,
                    op0=mybir.AluOpType.is_equal,
                )
            for j, off, sz, szp in TJ:
                g = psum([szp, D])
                for k, ksz in ((0, 128), (1, 68)):
                    nc.tensor.matmul(
                        g,
                        lhsT=ptg[:ksz, k, off : off + szp],
                        rhs=sh[:ksz, k, :],
                        start=(k == 0),
                        stop=(k == 1),
                    )
                nc.vector.tensor_add(
                    out=u0[:szp, b, j, :], in0=g, in1=pe_tm[:szp, j, :]
                )
                nc.vector.tensor_copy(
                    out=x_in[:szp, b, j, :], in_=u0[:szp, b, j, :]
                )

        def transpose_ap(in0, in1, xt, b):
            """DMA-transpose token-major slices ([128,512], [80,512]) of
            batch b into feature-major xt columns. High priority: these gate
            the downstream GEMM chains."""
            boff = b * NP
            with tc.high_priority():
                nc.scalar.dma_start(
                    out=xt[:, :, boff : boff + 128], in_=in0, transpose=True
                )
                nc.scalar.dma_start(
                    out=xt[:, :, boff + 128 : boff + 208], in_=in1, transpose=True
                )

        def transpose_b(xbf, xt, b):
            transpose_ap(xbf[:, b, 0, :], xbf[:80, b, 1, :], xt, b)

        def layernorm_chunk(u, out_tile, bs, pad_apply):
            """LN over the token tiles of batches `bs` (stats on real rows)."""
            nt = 2 * len(bs)
            bn_all = pst.tile([128, 4, 6], F32, tag="bn")
            mv_all = pst.tile([128, 4, 2], F32, tag="mv")
            nc.vector.memset(mv_all, 1.0)
            tl = [(b, j, off, sz, szp) for b in bs for (j, off, sz, szp) in TJ]
            for t, (b, j, off, sz, szp) in enumerate(tl):
                nc.vector.bn_stats(out=bn_all[:sz, t, :], in_=u[:sz, b, j, :])
                nc.vector.bn_aggr(out=mv_all[:sz, t, :], in_=bn_all[:sz, t, :])
            # rstd = (var + eps)^-0.5 via fast-rsqrt seed + 2 Newton steps,
            # all on DVE (keeps the ScalarE activation table untouched)
            v = pst.tile([128, 4], F32, tag="v")
            nc.vector.tensor_scalar(
                out=v[:, :nt],
                in0=mv_all[:, :nt, 1],
                scalar1=LN_EPS,
                scalar2=None,
                op0=mybir.AluOpType.add,
            )  # v = var + eps
            yi = pst.tile([128, 4], mybir.dt.int32, tag="yi")
            nc.vector.tensor_scalar(
                out=yi[:, :nt],
                in0=v[:, :nt].bitcast(mybir.dt.int32),
                scalar1=1,
                scalar2=None,
                op0=mybir.AluOpType.arith_shift_right,
            )
            nc.vector.tensor_tensor(
                out=yi[:, :nt],
                in0=rsq_k[:, :nt],
                in1=yi[:, :nt],
                op=mybir.AluOpType.subtract,
            )  # seed = magic - (bits(v) >> 1)
            rstd = plns.tile([128, 4], F32, tag="rstd")
            y = yi.bitcast(F32)
            t = pst.tile([128, 4], F32, tag="t")
            for it in range(2):
                src_y = y if it == 0 else rstd
                nc.vector.tensor_tensor(
                    out=t[:, :nt], in0=src_y[:, :nt], in1=src_y[:, :nt],
                    op=mybir.AluOpType.mult,
                )  # y^2
                nc.vector.tensor_tensor(
                    out=t[:, :nt], in0=t[:, :nt], in1=v[:, :nt],
                    op=mybir.AluOpType.mult,
                )  # v*y^2
                nc.vector.tensor_scalar(
                    out=t[:, :nt],
                    in0=t[:, :nt],
                    scalar1=-0.5,
                    scalar2=1.5,
                    op0=mybir.AluOpType.mult,
                    op1=mybir.AluOpType.add,
                )  # 1.5 - 0.5*v*y^2
                nc.vector.tensor_tensor(
                    out=rstd[:, :nt], in0=src_y[:, :nt], in1=t[:, :nt],
                    op=mybir.AluOpType.mult,
                )  # y *= (1.5 - 0.5*v*y^2)
            mrstd = plns.tile([128, 4], F32, tag="mrstd")
            nc.vector.tensor_tensor(
                out=mrstd[:, :nt],
                in0=mv_all[:, :nt, 0],
                in1=rstd[:, :nt],
                op=mybir.AluOpType.mult,
            )
            for t, (b, j, off, sz, szp) in enumerate(tl):
                asz = szp if pad_apply else sz
                nc.vector.tensor_scalar(
                    out=out_tile[:asz, b, j, :],
                    in0=u[:asz, b, j, :],
                    scalar1=mv_all[:asz, t, 0:1],
                    scalar2=rstd[:asz, t : t + 1],
                    op0=mybir.AluOpType.subtract,
                    op1=mybir.AluOpType.mult,
                )
            # per-batch scalar handles for f32 residual reconstruction
            return {b: (rstd, mrstd, 2 * bi) for bi, b in enumerate(bs)}

        def resid_ln(out_u, b, j, szp, ps, u_src, sc):
            """out_u[b,j] = ps + LN(u_src)[b,j] reconstructed in f32 from
            the stored per-tile (rstd, m*rstd) scalars."""
            rstd, mrstd, tb = sc
            t = tb + j
            nc.vector.tensor_scalar(
                out=out_u[:szp, b, j, :],
                in0=u_src[:szp, b, j, :],
                scalar1=rstd[:szp, t : t + 1],
                scalar2=mrstd[:szp, t : t + 1],
                op0=mybir.AluOpType.mult,
                op1=mybir.AluOpType.subtract,
            )
            nc.vector.tensor_tensor(
                out=out_u[:szp, b, j, :],
                in0=ps,
                in1=out_u[:szp, b, j, :],
                op=mybir.AluOpType.add,
            )

        def load_w_kt(w_dram, l, out_w, pool, tag):
            """4 contraction tiles [128, out_w] of a [D, out_w] weight."""
            ws = []
            for kt in range(4):
                w = pool.tile([128, out_w], BF16, tag=tag)
                nc.sync.dma_start(
                    out=w,
                    in_=_r(
                        w_dram.tensor,
                        l * D * out_w + kt * 128 * out_w,
                        [[out_w, 128], [1, out_w]],
                    ),
                )
                ws.append(w)
            return ws

        def load_qkv(l):
            return {
                "q": load_w_kt(Wq, l, D, pw, "w"),
                "k": load_w_kt(Wk, l, D, pw, "w"),
                "v": load_w_kt(Wv, l, D, pw, "w"),
            }

        def load_o_w1(l, w):
            w["o"] = load_w_kt(Wo, l, D, pw, "w")
            w["w1"] = load_w_kt(W1, l, FF, pw1, "w1")

        def load_w2(l, w):
            w["w2"] = []
            for fp in range(FF // 256):
                w2c = pw2.tile([128, 2, D], BF16, tag="w2")
                nc.sync.dma_start(
                    out=w2c,
                    in_=_r(
                        W2.tensor,
                        l * FF * D + fp * 256 * D,
                        [[D, 128], [128 * D, 2], [1, D]],
                    ),
                )
                w["w2"].append(w2c)

        def load_all(l):
            w = load_qkv(l)
            load_o_w1(l, w)
            load_w2(l, w)
            return w

        def gemm_units(xt, ws, o, cs):
            """Feature-major GEMM; one (dt, c) output block per unit."""
            for c in cs:
                for dt in range(4):
                    ps = psum([128, NCH])
                    for kt in range(4):
                        nc.tensor.matmul(
                            ps,
                            lhsT=ws[kt][:, dt * 128 : (dt + 1) * 128],
                            rhs=xt[:, kt, c * NCH : (c + 1) * NCH],
                            start=(kt == 0),
                            stop=(kt == 3),
                        )
                    nc.scalar.copy(out=o[:, dt, c * NCH : (c + 1) * NCH], in_=ps)
                    yield

        def v_units(xt, wvs, vt, bs):
            """V tiles (token-major, 33-col interleave); one (b,j) per unit."""
            for b in bs:
                for j, off, sz, szp in TJ:
                    ps = psum([sz, H, HD])
                    for kt in range(4):
                        nc.tensor.matmul(
                            ps,
                            lhsT=xt[:, kt, b * NP + off : b * NP + off + sz],
                            rhs=wvs[kt],
                            start=(kt == 0),
                            stop=(kt == 3),
                        )
                    nc.scalar.copy(out=vt[:sz, b, j, :, 0:HD], in_=ps)
                    yield

        def qk_exp_group(qt, kt_, b, g):
            """QK^T + paired exp for head group g of batch b -> 2 ptp tiles."""
            boff = b * NP
            ptps = []
            for ip in range(2):  # head pairs (2ip, 2ip+1)
                ptp = patt.tile([128, 2, 2, NP], BF16, tag="pt")
                for kj, koff, ksz in KJT:
                    spp = ppsp.tile([128, 2, 512], F32, tag="sp", name="spp")
                    for m in range(2):
                        i = 2 * ip + m
                        nc.tensor.matmul(
                            spp[:ksz, m, 0:NP],
                            lhsT=kt_[
                                32 * i : 32 * (i + 1),
                                g,
                                boff + koff : boff + koff + ksz,
                            ],
                            rhs=qt[32 * i : 32 * (i + 1), g, boff : boff + NP],
                            start=True,
                            stop=True,
                            tile_position=(32 * i, 0),
                        )
                    # exp of both heads in one op (banks are read-contiguous)
                    nc.scalar.activation(
                        out=ptp[:ksz, kj, :, :],
                        in_=spp[:ksz, :, 0:NP],
                        func=mybir.ActivationFunctionType.Exp,
                        scale=float(1.0 / np.sqrt(HD)),
                    )
                ptps.append(ptp)
            return ptps

        def av_group(vt, o_tm, b, g, ptps):
            """o^T = P^T V (token-major, ones-column denominator) + norm."""
            for qj, qoff, qsz in QJT:
                av8 = psum([qsz, 4, HD + 1])
                for i in range(4):
                    ip, m = divmod(i, 2)
                    h = 4 * g + i
                    for kj, koff, ksz in KJT:
                        nc.tensor.matmul(
                            av8[:, i, :],
                            lhsT=ptps[ip][:ksz, kj, m, qoff : qoff + qsz],
                            rhs=vt[:ksz, b, kj, h, :],
                            start=(kj == 0),
                            stop=(kj == 1),
                        )
                rc4 = psm.tile([128, 4], F32, tag="rc")
                nc.vector.reciprocal(out=rc4[:qsz], in_=av8[:, :, HD])
                for i in range(4):
                    h = 4 * g + i
                    nc.vector.tensor_scalar(
                        out=o_tm[:qsz, b, qj, h, :],
                        in0=av8[:, i, 0:HD],
                        scalar1=rc4[:qsz, i : i + 1],
                        scalar2=None,
                        op0=mybir.AluOpType.mult,
                    )

        attn_prev = [None]

        def attn_units(qt, kt_, vt, o_tm, ot, bs):
            for b in bs:
                for g in range(4):
                    ptps = qk_exp_group(qt, kt_, b, g)
                    p = attn_prev[0]
                    if p is not None:
                        av_group(vt, o_tm, p[0], p[1], p[2])
                        if p[1] == 3:
                            transpose_ap(
                                o_tm[:, p[0], 0, :, :],
                                o_tm[:80, p[0], 1, :, :],
                                ot,
                                p[0],
                            )
                    attn_prev[0] = (b, g, ptps)
                    yield

        def attn_drain(vt, o_tm, ot):
            p = attn_prev[0]
            av_group(vt, o_tm, p[0], p[1], p[2])
            if p[1] == 3:
                transpose_ap(
                    o_tm[:, p[0], 0, :, :], o_tm[:80, p[0], 1, :, :], ot, p[0]
                )
            attn_prev[0] = None

        def oproj_b(b, wos, ot, u, u_res, sc_res):
            for j, off, sz, szp in TJ:
                ps = psum([szp, D])
                for dt in range(4):
                    nc.tensor.matmul(
                        ps,
                        lhsT=ot[:, dt, b * NP + off : b * NP + off + szp],
                        rhs=wos[dt],
                        start=(dt == 0),
                        stop=(dt == 3),
                    )
                resid_ln(u, b, j, szp, ps, u_res, sc_res[b])

        def chunk_slices(c):
            csl = []
            for bi, b in enumerate((2 * c, 2 * c + 1)):
                for j, off, sz, szp in TJ:
                    csl.append((b, j, bi * NP + off, szp))
            return csl

        def ffn_units(w, xt2, b, u2, u_src, sc1):
            """FFN over one 208-token batch; FFN2 pipelined one f behind.
            Residual add emitted at drain so the held PSUM pair frees fast."""
            boff = b * NP
            osps = [
                pph.tile([szp, D], F32, tag="ps", name="pho")
                for (_, _, _, szp) in TJ
            ]
            hss = [None] * NF
            for f in range(NF):
                hp = psum([128, NP])
                for kt in range(4):
                    nc.tensor.matmul(
                        hp,
                        lhsT=w["w1"][kt][:, f * 128 : (f + 1) * 128],
                        rhs=xt2[:, kt, boff : boff + NP],
                        start=(kt == 0),
                        stop=(kt == 3),
                    )
                hs = ph.tile([128, NP], BF16, tag="h")
                nc.scalar.activation(
                    out=hs, in_=hp, func=mybir.ActivationFunctionType.Relu
                )
                hss[f] = hs
                if f > 0:
                    fp = f - 1
                    for si, (j, off, sz, szp) in enumerate(TJ):
                        nc.tensor.matmul(
                            osps[si],
                            lhsT=hss[fp][:, off : off + szp],
                            rhs=w["w2"][fp // 2][:, fp % 2, :],
                            start=(fp == 0),
                            stop=False,
                        )
                yield
            fp = NF - 1
            for si, (j, off, sz, szp) in enumerate(TJ):
                nc.tensor.matmul(
                    osps[si],
                    lhsT=hss[fp][:, off : off + szp],
                    rhs=w["w2"][fp // 2][:, fp % 2, :],
                    start=False,
                    stop=True,
                )
            for si, (j, off, sz, szp) in enumerate(TJ):
                resid_ln(u2, b, j, szp, osps[si], u_src, sc1[b])

        def chain(*gens):
            for g in gens:
                yield from g

        def new_qkv(xt, w, bs):
            """Allocate next-layer qt/kt/vt and emit gemm chunk0 + V bs."""
            qt = pqk.tile([128, 4, TCP], BF16, tag="qt")
            kt_ = pqk.tile([128, 4, TCP], BF16, tag="kt")
            vt = pqk.tile([128, BC, 2, H, HD + 1], BF16, tag="vt")
            nc.gpsimd.memset(vt[:, :, :, :, HD : HD + 1], 1.0)
            gen = chain(
                gemm_units(xt, w["q"], qt, [0]),
                gemm_units(xt, w["k"], kt_, [0]),
                v_units(xt, w["v"], vt, bs),
            )
            return qt, kt_, vt, gen

        # ---- layer 0 inputs (prologue tail) ----
        xt_cur = pxt.tile([128, 4, TCP], BF16, tag="xt")
        for b in range(BC):
            transpose_b(x_in, xt_cur, b)
        wcur = load_all(0)
        qt_c, kt_c, vt_c, gen0 = new_qkv(xt_cur, wcur, (0, 1))
        for _ in gen0:
            pass
        prev_u, prev_sc = u0, sc0

        trunc = meta.get("trunc", 99)

        def _trunc_out(src_tile):
            for b, j, off, sz, szp in TT:
                nc.scalar.dma_start(
                    out=y[b, off : off + sz, :], in_=src_tile[:sz, b, j, :]
                )

        if trunc == 0:
            _trunc_out(u0)
            return

        # ---- layers ----
        for l in range(L):
            last = l == L - 1
            wnext = load_qkv(l + 1) if not last else None
            o_tm = pqk.tile([128, BC, 2, H, HD], BF16, tag="otm")
            ot = pqk.tile([128, 4, TCP], BF16, tag="ot")

            # P0: gemm chunk1 + V b2,b3  ||  attention b0,b1
            _zip_emit(
                attn_units(qt_c, kt_c, vt_c, o_tm, ot, (0, 1)),
                (
                    chain(
                        gemm_units(xt_cur, wcur["q"], qt_c, [1]),
                        gemm_units(xt_cur, wcur["k"], kt_c, [1]),
                        v_units(xt_cur, wcur["v"], vt_c, (2, 3)),
                    ),
                    2,
                ),
            )

            if trunc == 1:
                _trunc_out(u0)
                return

            # P1: oproj c0 + LN1(c0) + FFN transpose c0
            x2 = pxbf.tile([128, BC, 2, D], BF16, tag="xbf")
            xt2 = pxt.tile([128, 4, TCP], BF16, tag="xt")
            u = px.tile([128, BC, 2, D], F32, tag="x")
            oproj_b(0, wcur["o"], ot, u, prev_u, prev_sc)
            attn_drain(vt_c, o_tm, ot)
            oproj_b(1, wcur["o"], ot, u, prev_u, prev_sc)
            sc1 = layernorm_chunk(u, x2, (0, 1), pad_apply=True)
            for b in (0, 1):
                transpose_b(x2, xt2, b)

            if trunc == 2:
                _trunc_out(u)
                return

            # P2: FFN b0,b1 || attention b2,b3
            if not last:
                load_o_w1(l + 1, wnext)
            u2 = px.tile([128, BC, 2, D], F32, tag="x")
            _zip_emit(
                attn_units(qt_c, kt_c, vt_c, o_tm, ot, (2, 3)),
                (chain(ffn_units(wcur, xt2, 0, u2, u, sc1),
                       ffn_units(wcur, xt2, 1, u2, u, sc1)), 4),
            )

            if trunc == 3:
                _trunc_out(u)
                return

            # P3: oproj c1 + LN1(c1) + T2(c1); LN2(c0) + Tnext(c0)
            oproj_b(2, wcur["o"], ot, u, prev_u, prev_sc)
            attn_drain(vt_c, o_tm, ot)
            oproj_b(3, wcur["o"], ot, u, prev_u, prev_sc)
            sc1b = layernorm_chunk(u, x2, (2, 3), pad_apply=True)
            sc1.update(sc1b)
            for b in (2, 3):
                transpose_b(x2, xt2, b)
            if last:
                x_next = px.tile([128, BC, 2, D], F32, tag="x")
            else:
                x_next = pxbf.tile([128, BC, 2, D], BF16, tag="xbf")
            xt_next = pxt.tile([128, 4, TCP], BF16, tag="xt")
            sc2 = layernorm_chunk(u2, x_next, (0, 1), pad_apply=not last)
            if not last:
                for b in (0, 1):
                    transpose_b(x_next, xt_next, b)

            if trunc == 4:
                _trunc_out(u2)
                return

            # P4: FFN b2,b3 || next-layer QK gemm c0 + V b0,b1
            if not last:
                load_w2(l + 1, wnext)
            streams = [(chain(ffn_units(wcur, xt2, 2, u2, u, sc1),
                              ffn_units(wcur, xt2, 3, u2, u, sc1)), 2)]
            if not last:
                qt_n, kt_n, vt_n, gen_n = new_qkv(xt_next, wnext, (0, 1))
                streams.append(gen_n)
            _zip_emit(*streams)

            if trunc == 5:
                _trunc_out(u2)
                return

            # P5: LN2(c1) + Tnext(c1)
            sc2b = layernorm_chunk(u2, x_next, (2, 3), pad_apply=not last)
            sc2.update(sc2b)
            if not last:
                for b in (2, 3):
                    transpose_b(x_next, xt_next, b)
                qt_c, kt_c, vt_c = qt_n, kt_n, vt_n
            prev_u, prev_sc = u2, sc2
            x_in = x_next
            xt_cur = xt_next
            wcur = wnext

        # ---- final LN over the last block output (f32 path, real rows) ----
        yt = px.tile([128, BC, 2, D], F32, tag="x")
        for c in range(2):
            layernorm_chunk(x_in, yt, (2 * c, 2 * c + 1), pad_apply=False)
        for b, j, off, sz, szp in TT:
            nc.scalar.dma_start(out=y[b, off : off + sz, :], in_=yt[:sz, b, j, :])


def _shapes(L):
    return {
        "xe": ([BC, 50, D], BF16),
        "idxf": ([BC, N], F32),
        "maskt": ([D], BF16),
        "pe": ([N, D], BF16),
        "iota2": ([128, 2], F32),
        "Wq": ([L, D, D], BF16),
        "Wk": ([L, D, D], BF16),
        "Wv": ([L, D, D], BF16),
        "Wo": ([L, D, D], BF16),
        "W1": ([L, D, FF], BF16),
        "W2": ([L, FF, D], BF16),
    }


def _build_nc(meta):
    nc = bacc.Bacc("TRN2", target_bir_lowering=False, debug=False, num_devices=N_CORES)
    ins = {}
    for name, (shape, dt) in _shapes(meta["L"]).items():
        ins[name] = nc.dram_tensor(name, list(shape), dt, kind="ExternalInput").ap()
    outs = {
        "y": nc.dram_tensor("y", [BC, N, D], F32, kind="ExternalOutput").ap()
    }
    with tile.TileContext(nc) as tc:
        build_decoder(tc, outs, ins, meta)
    nc.compile()
    return nc


def kernel(
    x_enc_out_vis,
    idx_restore_patches,
    mask_token,
    pos_emb,
    Wq, bq, Wk, bk, Wv, bv, Wo, bo,
    ln1_g, ln1_b,
    W1, b1, W2, b2,
    ln2_g, ln2_b,
    lnf_g, lnf_b,
):
    L = Wq.shape[0]

    # This instance of the model has all-zero biases and identity LN affine
    # params; the device program folds those away when true.
    def _zero(a):
        return not np.any(np.asarray(a))

    assert _zero(bq) and _zero(bk) and _zero(bv) and _zero(bo), (
        "nonzero attention biases not supported by this build"
    )
    assert _zero(b1) and _zero(b2), "nonzero FFN biases not supported"
    ln_gb = not (
        np.all(np.asarray(ln1_g) == 1.0)
        and _zero(ln1_b)
        and np.all(np.asarray(ln2_g) == 1.0)
        and _zero(ln2_b)
    )
    lnf_gb = not (np.all(np.asarray(lnf_g) == 1.0) and _zero(lnf_b))
    assert not ln_gb and not lnf_gb, "non-identity LN affine not supported"

    meta = {"L": L}
    nc = _build_nc(meta)

    f32 = np.float32
    bf16 = ml_dtypes.bfloat16

    def _bf(a):
        return np.ascontiguousarray(np.asarray(a, f32).astype(bf16))

    shared = {
        "maskt": _bf(np.asarray(mask_token, f32).reshape(D)),
        "pe": _bf(np.asarray(pos_emb, f32).reshape(N, D)),
        "iota2": np.stack(
            [np.arange(128, dtype=f32), np.arange(128, 256, dtype=f32)], axis=1
        ),
        "Wq": _bf(Wq),
        "Wk": _bf(Wk),
        "Wv": _bf(Wv),
        "Wo": _bf(Wo),
        "W1": _bf(W1),
        "W2": _bf(W2),
    }
    xe_np = np.asarray(x_enc_out_vis, f32).astype(bf16)
    idx_np = np.asarray(idx_restore_patches).astype(f32)
    in_maps = []
    for c in range(N_CORES):
        m = dict(shared)
        m["xe"] = np.ascontiguousarray(xe_np[c * BC : (c + 1) * BC])
        m["idxf"] = np.ascontiguousarray(idx_np[c * BC : (c + 1) * BC])
        in_maps.append(m)

    import time as _time
    _t0 = _time.time()
    res = run_bass_kernel_spmd(nc, in_maps, core_ids=list(range(N_CORES)))
    global _last_results, _last_exec_wall_s
    _last_exec_wall_s = _time.time() - _t0
    _last_results = res
    out = np.concatenate([r["y"] for r in res.results], axis=0)
    return out.astype(np.float32)


_last_results = None
_last_exec_wall_s = 0.0
